# Initial kernel scaffold
#
"""ComplexPolarAttention Trainium2 kernel.

score_ij = sum_d mag_i,d mag_j,d cos(phase_i,d - phase_j,d)
         = a_i . a_j + b_i . b_j          with a = mag*cos(phase), b = mag*sin(phase)
out_mag   = softmax(score, axis=1) @ mag
out_phase = softmax(score, axis=1) @ phase

Strategy (8 NeuronCores, SPMD, no collectives):
  - Rows (queries) sharded: core c owns queries [c*1024, (c+1)*1024).
  - Keys replicated: every core builds the full packed ab^T = [a|b]^T
    [128=2D, N] on-chip (ACT sin with per-partition bias for cos, DVE mul),
    so the two score GEMMs fuse into ONE K=128 fp32r matmul per key block.
  - Scores are computed transposed, S^T[k_blk=128, q=512] in PSUM, exp'd on
    ACT (scores bounded by D=64 < 88 so unnormalized exp cannot overflow),
    then used as the MOVING operand of the value matmuls whose stationary
    operands are natural-layout [mag|ones] (the ones column yields the
    softmax denominator for free) and phase.
  - PSUM accumulates the numerators over all 64 key blocks; the final
    divide by the denominator happens on host during the gather.
"""

import numpy as np
from contextlib import ExitStack

import concourse.bass as bass
import concourse.tile as tile
from concourse import bacc, mybir
from concourse.bass_utils import run_bass_kernel_spmd

F32 = mybir.dt.float32
F32R = mybir.dt.float32r
HALF_PI = float(np.pi / 2.0)


def build_program(n=8192, d=64, n_cores=8, qblk=512, enable_asserts=False):
    """Build the SPMD Bass program. Every core runs identical IR; per-core
    behavior comes only from per-core input data (the query slices)."""
    assert d == 64
    q = n // n_cores            # queries per core
    kblocks = n // 128          # key blocks of 128
    qblocks = q // qblk
    assert q % qblk == 0 and n % 128 == 0

    nc = bacc.Bacc(
        "TRN2",
        target_bir_lowering=False,
        debug=False,
        enable_asserts=enable_asserts,
        num_devices=n_cores,
    )

    # ---- DRAM I/O ----
    magt = nc.dram_tensor("magt", [d, n], F32, kind="ExternalInput").ap()
    phaset = nc.dram_tensor("phaset", [d, n], F32, kind="ExternalInput").ap()
    magt_q = nc.dram_tensor("magt_q", [d, q], F32, kind="ExternalInput").ap()
    phaset_q = nc.dram_tensor("phaset_q", [d, q], F32, kind="ExternalInput").ap()
    # [mag | ones] value matrix, pre-tiled on host to [128, kblocks*65]
    mo = nc.dram_tensor("mo", [128, kblocks * 65], F32, kind="ExternalInput").ap()
    # phase value matrix, pre-tiled on host to [128, kblocks*64]
    pv = nc.dram_tensor("pv", [128, kblocks * d], F32, kind="ExternalInput").ap()

    om = nc.dram_tensor("om", [65, q], F32, kind="ExternalOutput").ap()
    op = nc.dram_tensor("op", [d, q], F32, kind="ExternalOutput").ap()

    with tile.TileContext(nc) as tc, ExitStack() as ctx:
        const = ctx.enter_context(tc.tile_pool(name="const", bufs=1))
        persist = ctx.enter_context(tc.tile_pool(name="persist", bufs=1))
        bpool = ctx.enter_context(tc.tile_pool(name="build", bufs=3))
        epool = ctx.enter_context(tc.tile_pool(name="exps", bufs=4))
        opool = ctx.enter_context(tc.tile_pool(name="outs", bufs=2))
        spool = ctx.enter_context(tc.tile_pool(name="scores", bufs=3, space="PSUM"))
        apool = ctx.enter_context(tc.tile_pool(name="accum", bufs=1, space="PSUM"))

        # per-partition activation bias: pi/2 on partitions 0..63 (cos via
        # sin(x + pi/2)), 0 on partitions 64..127 (sin)
        bias_sc = const.tile([128, 1], F32)
        nc.vector.memset(bias_sc[0:64, :], HALF_PI)
        nc.vector.memset(bias_sc[64:128, :], 0.0)

        abt = persist.tile([128, n], F32)       # [a|b]^T for all keys
        abq = persist.tile([128, q], F32)       # [a|b]^T for this core's queries
        mo_t = persist.tile([128, kblocks, 65], F32)
        pv_t = persist.tile([128, kblocks, d], F32)

        nc.sync.dma_start(out=mo_t[:, :, :], in_=mo.rearrange("p (b m) -> p b m", m=65))
        nc.sync.dma_start(out=pv_t[:, :, :], in_=pv.rearrange("p (b m) -> p b m", m=d))

        # ---- build ab^T in chunks: ab[0:64] = mag*cos(phase), ab[64:128] = mag*sin(phase)
        def build_ab(dst, src_m, src_p, width, chunk):
            for c0 in range(0, width, chunk):
                sl = slice(c0, c0 + chunk)
                mg = bpool.tile([128, chunk], F32, tag="mg")
                nc.sync.dma_start(out=mg[0:64, :], in_=src_m[:, sl])
                nc.sync.dma_start(out=mg[64:128, :], in_=src_m[:, sl])
                ph = bpool.tile([128, chunk], F32, tag="ph")
                nc.sync.dma_start(out=ph[0:64, :], in_=src_p[:, sl])
                nc.sync.dma_start(out=ph[64:128, :], in_=src_p[:, sl])
                tr = bpool.tile([128, chunk], F32, tag="tr")
                nc.scalar.activation(
                    tr[:, :], ph[:, :], mybir.ActivationFunctionType.Sin,
                    bias=bias_sc[:, :], scale=1.0,
                )
                nc.vector.tensor_mul(dst[:, sl], mg[:, :], tr[:, :])

        build_ab(abq, magt_q, phaset_q, q, min(1024, q))
        build_ab(abt, magt, phaset, n, min(2048, n))

        # ---- main loop: for each q block, stream key blocks
        for qb in range(qblocks):
            qsl = slice(qb * qblk, (qb + 1) * qblk)
            psA = apool.tile([65, qblk], F32, tag="psA")   # mag numerators + den
            psB = apool.tile([64, qblk], F32, tag="psB")   # phase numerators
            es_prev = None
            for kb in range(kblocks):
                ss = spool.tile([128, qblk], F32)
                nc.tensor.matmul(
                    out=ss[:, :],
                    lhsT=abt[:, kb * 128:(kb + 1) * 128].bitcast(F32R),
                    rhs=abq[:, qsl].bitcast(F32R),
                    start=True, stop=True,
                )
                es = epool.tile([128, qblk], F32)
                nc.scalar.activation(
                    es[:, :], ss[:, :], mybir.ActivationFunctionType.Exp,
                )
                if es_prev is not None:
                    nc.tensor.matmul(
                        out=psA[:, :],
                        lhsT=mo_t[:, kb - 1, :].bitcast(F32R),
                        rhs=es_prev[:, :].bitcast(F32R),
                        start=(kb == 1), stop=False,
                    )
                    nc.tensor.matmul(
                        out=psB[:, :],
                        lhsT=pv_t[:, kb - 1, :].bitcast(F32R),
                        rhs=es_prev[:, :].bitcast(F32R),
                        start=(kb == 1), stop=False,
                    )
                es_prev = es
            nc.tensor.matmul(
                out=psA[:, :], lhsT=mo_t[:, kblocks - 1, :].bitcast(F32R),
                rhs=es_prev[:, :].bitcast(F32R), start=False, stop=True,
            )
            nc.tensor.matmul(
                out=psB[:, :], lhsT=pv_t[:, kblocks - 1, :].bitcast(F32R),
                rhs=es_prev[:, :].bitcast(F32R), start=False, stop=True,
            )
            oA = opool.tile([65, qblk], F32, tag="oA")
            nc.vector.tensor_copy(oA[:, :], psA[:, :])
            nc.sync.dma_start(out=om[:, qsl], in_=oA[:, :])
            oB = opool.tile([64, qblk], F32, tag="oB")
            nc.vector.tensor_copy(oB[:, :], psB[:, :])
            nc.sync.dma_start(out=op[:, qsl], in_=oB[:, :])

    nc.compile()
    return nc


def make_inputs(mag, phase, n_cores=8):
    """Host-side sharding/layout prep -> per-core input maps."""
    n, d = mag.shape
    q = n // n_cores
    kblocks = n // 128
    mag = np.ascontiguousarray(mag, dtype=np.float32)
    phase = np.ascontiguousarray(phase, dtype=np.float32)
    magt = np.ascontiguousarray(mag.T)
    phaset = np.ascontiguousarray(phase.T)
    mo = np.concatenate([mag, np.ones((n, 1), np.float32)], axis=1)
    mo = np.ascontiguousarray(
        mo.reshape(kblocks, 128, 65).transpose(1, 0, 2).reshape(128, -1))
    pv = np.ascontiguousarray(
        phase.reshape(kblocks, 128, d).transpose(1, 0, 2).reshape(128, -1))
    in_maps = []
    for c in range(n_cores):
        qsl = slice(c * q, (c + 1) * q)
        in_maps.append({
            "magt": magt,
            "phaset": phaset,
            "magt_q": np.ascontiguousarray(magt[:, qsl]),
            "phaset_q": np.ascontiguousarray(phaset[:, qsl]),
            "mo": mo,
            "pv": pv,
        })
    return in_maps


def gather_outputs(results, n, d, n_cores=8):
    """Per-core [65,q]/[64,q] transposed unnormalized sums -> full outputs."""
    new_mag = np.empty((n, d), np.float32)
    new_phase = np.empty((n, d), np.float32)
    q = n // n_cores
    for c in range(n_cores):
        om = results[c]["om"]          # [65, q]
        op = results[c]["op"]          # [64, q]
        den = om[64:65, :]             # [1, q]
        qsl = slice(c * q, (c + 1) * q)
        new_mag[qsl] = (om[:64, :] / den).T
        new_phase[qsl] = (op / den).T
    return new_mag, new_phase


_PROGRAM_CACHE = {}


def _get_program(n, d, n_cores):
    key = (n, d, n_cores)
    if key not in _PROGRAM_CACHE:
        _PROGRAM_CACHE[key] = build_program(n=n, d=d, n_cores=n_cores)
    return _PROGRAM_CACHE[key]


def kernel(mag, phase):
    mag = np.asarray(mag, dtype=np.float32)
    phase = np.asarray(phase, dtype=np.float32)
    n, d = mag.shape
    n_cores = 8
    nc = _get_program(n, d, n_cores)
    in_maps = make_inputs(mag, phase, n_cores=n_cores)
    res = run_bass_kernel_spmd(nc, in_maps, list(range(n_cores)))
    return gather_outputs(res.results, n, d, n_cores=n_cores)


# revision 8
# speedup vs baseline: 1.0453x; 1.0453x over previous
"""ComplexPolarAttention Trainium2 kernel.

score_ij = sum_d mag_i,d mag_j,d cos(phase_i,d - phase_j,d)
         = a_i . a_j + b_i . b_j          with a = mag*cos(phase), b = mag*sin(phase)
out_mag   = softmax(score, axis=1) @ mag
out_phase = softmax(score, axis=1) @ phase

Strategy (8 NeuronCores, SPMD, no collectives):
  - Rows (queries) sharded: core c owns queries [c*1024, (c+1)*1024).
  - Keys replicated: every core builds the full packed ab^T = [a|b]^T
    [128=2D, N] on-chip (ACT sin with per-partition bias for cos, DVE mul),
    so the two score GEMMs fuse into ONE K=128 fp32r matmul per key block.
  - Scores are computed transposed, S^T[k_blk=128, q=512] in PSUM, exp'd on
    ACT (scores bounded by D=64 < 88 so unnormalized exp cannot overflow),
    then used as the MOVING operand of the value matmuls whose stationary
    operands are natural-layout [mag|ones] (the ones column yields the
    softmax denominator for free) and phase.
  - PSUM accumulates the numerators over all 64 key blocks; the final
    divide by the denominator happens on host during the gather.
"""

import numpy as np
from contextlib import ExitStack

import concourse.bass as bass
import concourse.tile as tile
from concourse import bacc, mybir
from concourse.bass_utils import run_bass_kernel_spmd

F32 = mybir.dt.float32
F32R = mybir.dt.float32r
HALF_PI = float(np.pi / 2.0)


def build_program(n=8192, d=64, n_cores=8, qblk=512, enable_asserts=False):
    """Build the SPMD Bass program. Every core runs identical IR; per-core
    behavior comes only from per-core input data (the query slices)."""
    assert d == 64
    q = n // n_cores            # queries per core
    kblocks = n // 128          # key blocks of 128
    qblocks = q // qblk
    assert q % qblk == 0 and n % 128 == 0

    nc = bacc.Bacc(
        "TRN2",
        target_bir_lowering=False,
        debug=False,
        enable_asserts=enable_asserts,
        num_devices=n_cores,
    )

    # ---- DRAM I/O ----
    # ph2: [wrap(phase^T + pi/2) | wrap(phase^T)] stacked to 128 partitions,
    # wrapped into [-pi, pi) on host (ScalarE Sin domain); sin of the top
    # half gives cos(phase), of the bottom half sin(phase).
    magt = nc.dram_tensor("magt", [d, n], F32, kind="ExternalInput").ap()
    ph2 = nc.dram_tensor("ph2", [2 * d, n], F32, kind="ExternalInput").ap()
    magt_q = nc.dram_tensor("magt_q", [d, q], F32, kind="ExternalInput").ap()
    ph2_q = nc.dram_tensor("ph2_q", [2 * d, q], F32, kind="ExternalInput").ap()
    # [mag | ones] value matrix, pre-tiled on host to [128, kblocks*65]
    mo = nc.dram_tensor("mo", [128, kblocks * 65], F32R, kind="ExternalInput").ap()
    # phase value matrix, pre-tiled on host to [128, kblocks*64]
    pv = nc.dram_tensor("pv", [128, kblocks * d], F32R, kind="ExternalInput").ap()

    om = nc.dram_tensor("om", [65, q], F32, kind="ExternalOutput").ap()
    op = nc.dram_tensor("op", [d, q], F32, kind="ExternalOutput").ap()

    with tile.TileContext(nc) as tc, ExitStack() as ctx:
        const = ctx.enter_context(tc.tile_pool(name="const", bufs=1))
        persist = ctx.enter_context(tc.tile_pool(name="persist", bufs=1))
        bpool = ctx.enter_context(tc.tile_pool(name="build", bufs=3))
        epool = ctx.enter_context(tc.tile_pool(name="exps", bufs=4))
        opool = ctx.enter_context(tc.tile_pool(name="outs", bufs=2))
        spool = ctx.enter_context(tc.tile_pool(name="scores", bufs=3, space="PSUM"))
        apool = ctx.enter_context(tc.tile_pool(name="accum", bufs=1, space="PSUM"))

        abt = persist.tile([128, n], F32R)       # [a|b]^T for all keys
        abq = persist.tile([128, q], F32R)       # [a|b]^T for this core's queries
        mo_t = persist.tile([128, kblocks, 65], F32R)
        pv_t = persist.tile([128, kblocks, d], F32R)

        nc.sync.dma_start(out=mo_t[:, :, :], in_=mo.rearrange("p (b m) -> p b m", m=65))
        nc.sync.dma_start(out=pv_t[:, :, :], in_=pv.rearrange("p (b m) -> p b m", m=d))

        # ---- build ab^T in chunks: ab[0:64] = mag*cos(phase), ab[64:128] = mag*sin(phase)
        def build_ab(dst, src_m, src_p2, width, chunk):
            for c0 in range(0, width, chunk):
                sl = slice(c0, c0 + chunk)
                mg = bpool.tile([128, chunk], F32, tag="mg")
                nc.sync.dma_start(out=mg[0:64, :], in_=src_m[:, sl])
                nc.sync.dma_start(out=mg[64:128, :], in_=src_m[:, sl])
                ph = bpool.tile([128, chunk], F32, tag="ph")
                nc.sync.dma_start(out=ph[:, :], in_=src_p2[:, sl])
                tr = bpool.tile([128, chunk], F32, tag="tr")
                nc.scalar.activation(
                    tr[:, :], ph[:, :], mybir.ActivationFunctionType.Sin,
                )
                nc.vector.tensor_mul(dst[:, sl], mg[:, :], tr[:, :])

        build_ab(abq, magt_q, ph2_q, q, min(1024, q))
        build_ab(abt, magt, ph2, n, min(2048, n))

        # ---- main loop: for each q block, stream key blocks in PAIRS
        # (two score matmuls land in one 2-bank PSUM tile so a single wide
        # exp amortizes ACT's ~352-cycle per-instruction overhead)
        assert kblocks % 2 == 0
        for qb in range(qblocks):
            qsl = slice(qb * qblk, (qb + 1) * qblk)
            psA = apool.tile([65, qblk], F32, tag="psA")   # mag numerators + den
            psB = apool.tile([64, qblk], F32, tag="psB")   # phase numerators

            def value_mms(es, kp, first, last):
                for j in range(2):
                    kb = 2 * kp + j
                    esl = es[:, j * qblk:(j + 1) * qblk]
                    nc.tensor.matmul(
                        out=psA[:, :], lhsT=mo_t[:, kb, :], rhs=esl,
                        start=(first and j == 0), stop=(last and j == 1),
                    )
                    nc.tensor.matmul(
                        out=psB[:, :], lhsT=pv_t[:, kb, :], rhs=esl,
                        start=(first and j == 0), stop=(last and j == 1),
                    )

            es_prev = None
            for kp in range(kblocks // 2):
                ss = spool.tile([128, 2 * qblk], F32)
                for j in range(2):
                    nc.tensor.matmul(
                        out=ss[:, j * qblk:(j + 1) * qblk],
                        lhsT=abt[:, (2 * kp + j) * 128:(2 * kp + j + 1) * 128],
                        rhs=abq[:, qsl],
                        start=True, stop=True,
                    )
                es = epool.tile([128, 2 * qblk], F32R)
                nc.scalar.activation(
                    es[:, :], ss[:, :], mybir.ActivationFunctionType.Exp,
                )
                if es_prev is not None:
                    value_mms(es_prev, kp - 1, first=(kp == 1), last=False)
                es_prev = es
            value_mms(es_prev, kblocks // 2 - 1, first=False, last=True)
            oA = opool.tile([65, qblk], F32, tag="oA")
            nc.vector.tensor_copy(oA[:, :], psA[:, :])
            nc.sync.dma_start(out=om[:, qsl], in_=oA[:, :])
            oB = opool.tile([64, qblk], F32, tag="oB")
            nc.vector.tensor_copy(oB[:, :], psB[:, :])
            nc.sync.dma_start(out=op[:, qsl], in_=oB[:, :])

    nc.compile()
    return nc


def make_inputs(mag, phase, n_cores=8):
    """Host-side sharding/layout prep -> per-core input maps."""
    n, d = mag.shape
    q = n // n_cores
    kblocks = n // 128
    mag = np.ascontiguousarray(mag, dtype=np.float32)
    phase = np.ascontiguousarray(phase, dtype=np.float32)
    magt = np.ascontiguousarray(mag.T)

    def wrap(x):
        # into [-pi, pi) -- ScalarE Sin domain
        return ((x + np.pi) % (2.0 * np.pi) - np.pi).astype(np.float32)

    ph2 = np.concatenate([wrap(phase.T + HALF_PI), wrap(phase.T)], axis=0)
    ph2 = np.ascontiguousarray(ph2)
    mo = np.concatenate([mag, np.ones((n, 1), np.float32)], axis=1)
    mo = np.ascontiguousarray(
        mo.reshape(kblocks, 128, 65).transpose(1, 0, 2).reshape(128, -1))
    pv = np.ascontiguousarray(
        phase.reshape(kblocks, 128, d).transpose(1, 0, 2).reshape(128, -1))
    in_maps = []
    for c in range(n_cores):
        qsl = slice(c * q, (c + 1) * q)
        in_maps.append({
            "magt": magt,
            "ph2": ph2,
            "magt_q": np.ascontiguousarray(magt[:, qsl]),
            "ph2_q": np.ascontiguousarray(ph2[:, qsl]),
            "mo": mo,
            "pv": pv,
        })
    return in_maps


def gather_outputs(results, n, d, n_cores=8):
    """Per-core [65,q]/[64,q] transposed unnormalized sums -> full outputs."""
    new_mag = np.empty((n, d), np.float32)
    new_phase = np.empty((n, d), np.float32)
    q = n // n_cores
    for c in range(n_cores):
        om = results[c]["om"]          # [65, q]
        op = results[c]["op"]          # [64, q]
        den = om[64:65, :]             # [1, q]
        qsl = slice(c * q, (c + 1) * q)
        new_mag[qsl] = (om[:64, :] / den).T
        new_phase[qsl] = (op / den).T
    return new_mag, new_phase


_PROGRAM_CACHE = {}


def _get_program(n, d, n_cores):
    key = (n, d, n_cores)
    if key not in _PROGRAM_CACHE:
        _PROGRAM_CACHE[key] = build_program(n=n, d=d, n_cores=n_cores)
    return _PROGRAM_CACHE[key]


def kernel(mag, phase):
    mag = np.asarray(mag, dtype=np.float32)
    phase = np.asarray(phase, dtype=np.float32)
    n, d = mag.shape
    n_cores = 8
    nc = _get_program(n, d, n_cores)
    in_maps = make_inputs(mag, phase, n_cores=n_cores)
    res = run_bass_kernel_spmd(nc, in_maps, list(range(n_cores)))
    return gather_outputs(res.results, n, d, n_cores=n_cores)
